# revision 9
# baseline (speedup 1.0000x reference)
"""Trainium2 Bass kernel for nn_DecoderWithAttention (show-attend-tell decoder).

Strategy (8 NeuronCores, data-parallel over batch):
  - Host: stable argsort by caption length (desc), deal 8 contiguous examples
    to each core; precompute the x-side (teacher-forced embedding) GRU/proj
    contributions for all timesteps as one batched GEMM on host; pack weights
    as lhsT tiles.
  - Device (per core, batch=8, T=159 recurrent steps, feature-on-partition
    layout [128p, (tile, batch)f], bf16 matmuls with fp32 accumulation):
    attention (PE + ACT/DVE, softmax without max-subtraction, 1/sum folded
    into the gate multiply), 2 GRU cells + 2 LayerNorms per step (LN stats
    via ones-matmuls), h2res streamed to DRAM; then a batched FFN->vocab
    phase over all (t, b) rows.
  - Host: normalize alphas (exp/sum), zero masked rows, undo ordering.
"""

import numpy as np

T = 159
T_PAD = 160
B = 64
P = 196
ENC = 512
DEC = 512
EMB = 256
ATT = 128
FFN = 1024
V = 2000
NB = 8          # examples per core
NC_CORES = 8
PH = 98         # P / 2 (per-half patch rows)
EPS = 1e-5

_CACHE = {}


# ---------------------------------------------------------------- fix waits
def _split_multi_waits(nc, max_waits=1):
    """walrus in this container encodes at most ONE sync wait per
    instruction; hoist extra waits onto preceding same-engine NoOps."""
    from concourse import mybir
    n = 0
    ctr = [0]
    for f in nc.m.functions:
        for blk in f.blocks:
            out = []
            changed = False
            for ins in blk.instructions:
                si = ins.sync_info
                waits = list(si.on_wait) if (si and si.on_wait) else []
                if len(waits) > max_waits:
                    for w in waits[:-max_waits]:
                        ctr[0] += 1
                        out.append(mybir.InstNoOp(
                            name=f"waitfix-{id(nc)}-{ctr[0]}",
                            engine=ins.engine,
                            sync_info=mybir.SyncInfo(on_wait=[w], on_update=[]),
                        ))
                        n += 1
                    si.on_wait = waits[-max_waits:]
                    changed = True
                out.append(ins)
            if changed:
                blk.instructions = out
    return n


# ---------------------------------------------------------------- builder
def _build(t_steps):
    import concourse.bass as bass
    import concourse.tile as tile
    from concourse import mybir
    from contextlib import ExitStack

    f32 = mybir.dt.float32
    bf16 = mybir.dt.bfloat16
    AF = mybir.ActivationFunctionType
    OP = mybir.AluOpType

    nc = bass.Bass("TRN2")

    def din(name, shape, dt=bf16):
        return nc.dram_tensor(name, shape, dt, kind="ExternalInput")

    # --- DRAM tensors
    encT = din("encT", [512, NB * P])              # [ENC, (b,p)]
    encP = din("encP", [PH, NB * 2 * ENC])         # [98, (b, half, enc)]
    attw = din("attw", [128, 4 * ATT])             # dec_att lhsT (kt-major)
    gatew1 = din("gatew1", [128, 4 * ATT])
    wiha = din("wiha", [128, 4 * 3 * DEC])
    whh1 = din("whh1", [128, 4 * 3 * DEC])
    wih2 = din("wih2", [128, 4 * 3 * DEC])
    whh2 = din("whh2", [128, 4 * 3 * DEC])
    proja = din("proja", [128, 4 * DEC])
    encattw = din("encattw", [128, 4 * ATT])
    ffn1w = din("ffn1w", [128, 4 * FFN])
    ffn2w = din("ffn2w", [128, 8 * V])
    wfull = din("wfull", [128, 1])
    gatew2 = din("gatew2", [128, 1])

    dec_att_b = din("dec_att_b", [128, 1], f32)
    enc_att_b = din("enc_att_b", [128, 1], f32)
    gate_b1 = din("gate_b1", [128, 1], f32)
    gate_b2 = din("gate_b2", [1, 1], f32)
    bhh1n = din("bhh1n", [128, 4], f32)
    brz2 = din("brz2", [128, 8], f32)
    bign2 = din("bign2", [128, 4], f32)
    bhh2n = din("bhh2n", [128, 4], f32)
    ln1g = din("ln1g", [128, 4], f32)
    ln1b = din("ln1b", [128, 4], f32)
    ln2g = din("ln2g", [128, 4], f32)
    ln2b = din("ln2b", [128, 4], f32)

    ffn1b = din("ffn1b", [128, 8], f32)
    ffn2b = din("ffn2b", [1, 2 * V])
    gi1x = din("gi1x", [T_PAD, 128, 96], f32)
    projx = din("projx", [T_PAD, 128, 32], f32)

    preds = nc.dram_tensor("preds", [T_PAD * NB, V], f32, kind="ExternalOutput")
    exout = nc.dram_tensor("exout", [T_PAD, PH, 16], bf16, kind="ExternalOutput")
    sums = nc.dram_tensor("sums", [1, T_PAD * NB], f32, kind="ExternalOutput")
    h2res_d = nc.dram_tensor("h2res_d", [T_PAD, 128, 32], bf16, kind="Internal")

    def bc(ap, dims):
        """Rebuild AP with given free dims (list of [step, count])."""
        return bass.AP(tensor=ap.tensor, offset=ap.offset, ap=[ap.ap[0]] + dims)

    with tile.TileContext(nc) as tc, ExitStack() as ctx:
        wpool = ctx.enter_context(tc.tile_pool(name="w", bufs=1))
        state = ctx.enter_context(tc.tile_pool(name="st", bufs=1))

        # --- load weights/biases into SBUF
        def wload(dram, shape, dt=bf16):
            t_ = wpool.tile(shape, dt, tag=dram.name)
            nc.sync.dma_start(out=t_, in_=dram[:, :])
            return t_

        attw_s = wload(attw, [128, 4 * ATT])
        gatew1_s = wload(gatew1, [128, 4 * ATT])
        wiha_s = wload(wiha, [128, 4 * 3 * DEC])
        whh1_s = wload(whh1, [128, 4 * 3 * DEC])
        wih2_s = wload(wih2, [128, 4 * 3 * DEC])
        whh2_s = wload(whh2, [128, 4 * 3 * DEC])
        proja_s = wload(proja, [128, 4 * DEC])
        ffn1w_s = wload(ffn1w, [128, 4 * FFN])
        ffn2w_s = wload(ffn2w, [128, 8 * V])
        wfull_s = wload(wfull, [128, 1])
        gatew2_s = wload(gatew2, [128, 1])
        encP_s = wload(encP, [PH, NB * 2 * ENC])

        datb_s = wload(dec_att_b, [128, 1], f32)
        gb1_s = wload(gate_b1, [128, 1], f32)
        gb2_s = wload(gate_b2, [1, 1], f32)
        bhh1n_s = wload(bhh1n, [128, 4], f32)
        brz2_s = wload(brz2, [128, 8], f32)
        bign2_s = wload(bign2, [128, 4], f32)
        bhh2n_s = wload(bhh2n, [128, 4], f32)
        ln1g_s = wload(ln1g, [128, 4], f32)
        ln1b_s = wload(ln1b, [128, 4], f32)
        ln2g_s = wload(ln2g, [128, 4], f32)
        ln2b_s = wload(ln2b, [128, 4], f32)
        eatb_s = wload(enc_att_b, [128, 1], f32)
        ffn1b_s = wload(ffn1b, [128, 8], f32)
        ffn2b_s = wload(ffn2b, [1, 2 * V])

        ones_bf = state.tile([128, 1], bf16)
        nc.vector.memset(ones_bf, 1.0)
        ones_f = state.tile([128, 1], f32)
        nc.vector.memset(ones_f, 1.0)
        ones_r = state.tile([1, 128], f32)
        nc.vector.memset(ones_r, 1.0)
        ones_rb = state.tile([1, 128], bf16)
        nc.vector.memset(ones_rb, 1.0)
        eps_s = state.tile([1, 1], f32)
        nc.vector.memset(eps_s, EPS)

        h1 = state.tile([128, 32], f32)
        nc.vector.memset(h1, 0.0)
        h2 = state.tile([128, 32], f32)
        nc.vector.memset(h2, 0.0)
        h1b = state.tile([128, 32], bf16)
        nc.vector.memset(h1b, 0.0)
        h2b = state.tile([128, 32], bf16)
        nc.vector.memset(h2b, 0.0)
        sums_s = state.tile([1, T_PAD * NB], f32)
        nc.vector.memset(sums_s, 1.0)
        att1_s = state.tile([128, NB * P], bf16)

        zer32 = state.tile([128, 32], bf16)
        nc.vector.memset(zer32, 0.0)
        nc.sync.dma_start(out=h2res_d[t_steps, :, :], in_=zer32)
        if t_steps + 1 < T_PAD:
            for tz in range(t_steps + 1, T_PAD):
                nc.sync.dma_start(out=h2res_d[tz, :, :], in_=zer32)

        # --- prologue: att1 = enc @ enc_att_w.T + b  -> [ATT, (b, p)] bf16
        with tc.tile_pool(name="pro", bufs=2) as pro, \
             tc.tile_pool(name="props", bufs=2, space="PSUM") as props:
            encattw_s = wload(encattw, [128, 4 * ATT])
            encT_s = pro.tile([128, 4 * NB * P], bf16, tag="encT")
            for kt in range(4):
                nc.sync.dma_start(
                    out=encT_s[:, kt * NB * P:(kt + 1) * NB * P],
                    in_=encT[kt * 128:(kt + 1) * 128, :])
            for b_ in range(NB):
                a1p = props.tile([128, P], f32, tag="a1p")
                for kt in range(4):
                    nc.tensor.matmul(
                        a1p,
                        encattw_s[:, kt * ATT:(kt + 1) * ATT],
                        encT_s[:, kt * NB * P + b_ * P: kt * NB * P + (b_ + 1) * P],
                        start=(kt == 0), stop=(kt == 3))
                nc.scalar.activation(
                    out=att1_s[:, b_ * P:(b_ + 1) * P], in_=a1p,
                    func=AF.Identity, bias=eatb_s)

        # --- recurrent steps
        with tc.tile_pool(name="sp", bufs=3) as sp, \
             tc.tile_pool(name="gx", bufs=3) as gx, \
             tc.tile_pool(name="pstep", bufs=1, space="PSUM") as pstep:
            pb1 = [pstep.tile([128, 512], f32, tag="pb1a", name="pb1a"),
                   pstep.tile([128, 512], f32, tag="pb1b", name="pb1b")]
            pb2 = [pstep.tile([128, 512], f32, tag="pb2a", name="pb2a"),
                   pstep.tile([128, 512], f32, tag="pb2b", name="pb2b")]
            pb3 = pstep.tile([128, 512], f32, tag="pb3", name="pb3")
            pb4 = pstep.tile([128, 512], f32, tag="pb4", name="pb4")
            pb5 = pstep.tile([128, 512], f32, tag="pb5", name="pb5")

            def mm_gru(p, w_s, rhs_s, is_i_side):
                # rz part (mt 0..7) accumulates into p[:,0:64]; n part into
                # ig [64:96] (i-side) or hg [96:128] (h-side)
                for mt in range(12):
                    if mt < 8:
                        off = mt * NB
                        first = not is_i_side
                    elif is_i_side:
                        off = 64 + (mt - 8) * NB
                        first = True
                    else:
                        off = 96 + (mt - 8) * NB
                        first = True
                    for kt in range(4):
                        nc.tensor.matmul(
                            p[:, off:off + NB],
                            w_s[:, kt * 12 * 128 + mt * 128:
                                kt * 12 * 128 + (mt + 1) * 128],
                            rhs_s[:, kt * NB:(kt + 1) * NB],
                            start=(kt == 0 and first),
                            stop=(kt == 3 and (is_i_side or mt >= 8)))

            def mm_ws(out_ps, w_s, rhs_s, kts, mts, n=NB):
                for mt in range(mts):
                    for kt in range(kts):
                        nc.tensor.matmul(
                            out_ps[:, mt * n:(mt + 1) * n],
                            w_s[:, kt * mts * 128 + mt * 128:
                                kt * mts * 128 + (mt + 1) * 128],
                            rhs_s[:, kt * n:(kt + 1) * n],
                            start=(kt == 0), stop=(kt == kts - 1))

            def layer_norm(xin, g_s, b_s, out_bf, out_f32, tagp, stats_off):
                s1 = pb4[0:1, stats_off:stats_off + 32]
                s2 = pb4[0:1, stats_off + 32:stats_off + 64]
                mrb = pb4[:, stats_off + 64:stats_off + 80]
                sq = sp.tile([128, 32], f32, tag=tagp + "sq", name=tagp + "sq")
                nc.vector.tensor_mul(out=sq, in0=xin, in1=xin)
                nc.tensor.matmul(s1, ones_f, xin, start=True, stop=True)
                nc.tensor.matmul(s2, ones_f, sq, start=True, stop=True)
                mr = sp.tile([1, 16], f32, tag=tagp + "mr", name=tagp + "mr")
                nc.vector.tensor_reduce(
                    out=mr[0:1, 0:8],
                    in_=bass.AP(tensor=s1.tensor, offset=s1.offset,
                                ap=[s1.ap[0], [1, 8], [8, 4]]),
                    axis=mybir.AxisListType.X, op=OP.add)
                nc.vector.tensor_reduce(
                    out=mr[0:1, 8:16],
                    in_=bass.AP(tensor=s2.tensor, offset=s2.offset,
                                ap=[s2.ap[0], [1, 8], [8, 4]]),
                    axis=mybir.AxisListType.X, op=OP.add)
                nc.vector.tensor_scalar_mul(out=mr[0:1, 0:8],
                                            in0=mr[0:1, 0:8], scalar1=1.0 / DEC)
                msq = sp.tile([1, 8], f32, tag=tagp + "msq", name=tagp + "msq")
                nc.vector.tensor_mul(out=msq, in0=mr[0:1, 0:8],
                                     in1=mr[0:1, 0:8])
                nc.vector.tensor_scalar_mul(out=mr[0:1, 8:16],
                                            in0=mr[0:1, 8:16], scalar1=1.0 / DEC)
                nc.vector.tensor_sub(out=mr[0:1, 8:16], in0=mr[0:1, 8:16],
                                     in1=msq)
                sd = sp.tile([1, 8], f32, tag=tagp + "sd", name=tagp + "sd")
                nc.scalar.activation(out=sd, in_=mr[0:1, 8:16], func=AF.Sqrt,
                                     bias=eps_s)
                nc.vector.reciprocal(out=mr[0:1, 8:16], in_=sd)
                nc.tensor.matmul(mrb, ones_r, mr, start=True, stop=True)
                t1 = sp.tile([128, 32], f32, tag=tagp + "t1", name=tagp + "t1")
                nc.vector.tensor_tensor(
                    out=t1, in0=xin,
                    in1=bc(mrb[:, 0:8], [[0, 4], [1, 8]]), op=OP.subtract)
                nc.vector.tensor_tensor(
                    out=t1, in0=t1,
                    in1=bc(mrb[:, 8:16], [[0, 4], [1, 8]]), op=OP.mult)
                nc.vector.tensor_tensor(
                    out=t1, in0=t1,
                    in1=bc(g_s[:, :], [[1, 4], [0, 8]]), op=OP.mult)
                if out_f32 is not None:
                    nc.vector.tensor_tensor(
                        out=out_f32, in0=t1,
                        in1=bc(b_s[:, :], [[1, 4], [0, 8]]), op=OP.add)
                nc.vector.tensor_tensor(
                    out=out_bf, in0=t1,
                    in1=bc(b_s[:, :], [[1, 4], [0, 8]]), op=OP.add)

            for t in range(t_steps):
                pA = pb1[t % 2]
                pB = pb2[t % 2]
                rz1_ps = pA[:, 0:64]
                ig1_ps = pA[:, 64:96]
                hg1_ps = pA[:, 96:128]
                rz2_ps = pB[:, 0:64]
                ig2_ps = pB[:, 64:96]
                hg2_ps = pB[:, 96:128]
                att2_ps = pb3[:, 0:8]
                g1_ps = pb3[:, 8:16]
                et0 = pb3[0:98, 16:24]
                et1 = pb3[0:98, 24:32]
                sum_ps = pb3[0:1, 32:40]
                g2_ps = pb3[0:1, 48:56]
                grn_ps = pb3[:, 56:64]
                awe_ps = pb5[:, 0:32]
                proj_ps = pb5[:, 32:64]

                gi1x_s = gx.tile([128, 96], f32, tag="gi1x", name="gi1x_s")
                nc.sync.dma_start(out=gi1x_s, in_=gi1x[t, :, :])
                projx_s = gx.tile([128, 32], f32, tag="projx", name="projx_s")
                nc.sync.dma_start(out=projx_s, in_=projx[t, :, :])

                # attention scores + gate first matmul (PE), then h-side GRU
                mm_ws(att2_ps, attw_s, h2b, 4, 1)
                mm_ws(g1_ps, gatew1_s, h2b, 4, 1)
                mm_gru(pB, whh2_s, h2b, False)
                mm_gru(pA, whh1_s, h1b, False)

                att2b = sp.tile([128, NB], f32, tag="att2b", name="att2b")
                nc.scalar.activation(out=att2b, in_=att2_ps, func=AF.Identity,
                                     bias=datb_s)
                y_s = sp.tile([128, NB * P], bf16, tag="y", name="y_s")
                for b_ in range(NB):
                    if b_ % 2 == 0:
                        nc.scalar.activation(
                            out=y_s[:, b_ * P:(b_ + 1) * P],
                            in_=att1_s[:, b_ * P:(b_ + 1) * P],
                            func=AF.Relu, bias=att2b[:, b_:b_ + 1])
                    else:
                        nc.vector.tensor_scalar(
                            out=y_s[:, b_ * P:(b_ + 1) * P],
                            in0=att1_s[:, b_ * P:(b_ + 1) * P],
                            scalar1=att2b[:, b_:b_ + 1], scalar2=0.0,
                            op0=OP.add, op1=OP.max)

                for b_ in range(NB):
                    nc.tensor.matmul(
                        et0[:, b_:b_ + 1],
                        y_s[:, b_ * P: b_ * P + PH], wfull_s,
                        start=True, stop=True)
                    nc.tensor.matmul(
                        et1[:, b_:b_ + 1],
                        y_s[:, b_ * P + PH: b_ * P + 2 * PH], wfull_s,
                        start=True, stop=True)

                ex_s = sp.tile([PH, 16], bf16, tag="ex", name="ex_s")
                nc.scalar.activation(out=ex_s[:, 0:8], in_=et0, func=AF.Exp)
                nc.scalar.activation(out=ex_s[:, 8:16], in_=et1, func=AF.Exp)
                nc.sync.dma_start(out=exout[t, :, :], in_=ex_s)

                nc.tensor.matmul(sum_ps, ones_bf[0:PH, :], ex_s[0:PH, 0:8],
                                 start=True, stop=False)
                nc.tensor.matmul(sum_ps, ones_bf[0:PH, :], ex_s[0:PH, 8:16],
                                 start=False, stop=True)
                nc.vector.tensor_copy(out=sums_s[0:1, t * NB:(t + 1) * NB],
                                      in_=sum_ps)
                recip = sp.tile([1, NB], f32, tag="recip", name="recip")
                nc.vector.reciprocal(out=recip,
                                     in_=sums_s[0:1, t * NB:(t + 1) * NB])

                grelu = sp.tile([128, NB], bf16, tag="grelu", name="grelu")
                nc.scalar.activation(out=grelu, in_=g1_ps, func=AF.Relu,
                                     bias=gb1_s)
                nc.tensor.matmul(g2_ps, gatew2_s, grelu, start=True, stop=True)
                gate = sp.tile([1, NB], f32, tag="gate", name="gate")
                nc.scalar.activation(out=gate, in_=g2_ps, func=AF.Sigmoid,
                                     bias=gb2_s)
                grn = sp.tile([1, NB], f32, tag="grn", name="grn")
                nc.vector.tensor_mul(out=grn, in0=gate, in1=recip)
                nc.tensor.matmul(grn_ps, ones_r, grn, start=True, stop=True)
                grnb = sp.tile([128, NB], f32, tag="grnb", name="grnb")
                nc.scalar.activation(out=grnb, in_=grn_ps, func=AF.Identity)

                for b_ in range(NB):
                    for et in range(4):
                        for hf in range(2):
                            nc.tensor.matmul(
                                awe_ps[:, et * NB + b_: et * NB + b_ + 1],
                                encP_s[:, b_ * 1024 + hf * 512 + et * 128:
                                       b_ * 1024 + hf * 512 + (et + 1) * 128],
                                ex_s[:, hf * 8 + b_: hf * 8 + b_ + 1],
                                start=(hf == 0), stop=(hf == 1))
                awe_s = sp.tile([128, 32], bf16, tag="awe", name="awe_s")
                nc.vector.tensor_tensor(
                    out=awe_s, in0=awe_ps,
                    in1=bc(grnb[:, 0:NB], [[0, 4], [1, NB]]),
                    op=OP.mult)

                # GRU1
                mm_gru(pA, wiha_s, awe_s, True)

                trz = sp.tile([128, 64], f32, tag="trz", name="trz")
                nc.vector.tensor_add(out=trz, in0=rz1_ps, in1=gi1x_s[:, 0:64])
                rz = sp.tile([128, 64], f32, tag="rz", name="rz")
                nc.scalar.activation(out=rz, in_=trz, func=AF.Sigmoid)
                tn = sp.tile([128, 32], f32, tag="tn", name="tn")
                nc.vector.tensor_tensor(
                    out=tn, in0=hg1_ps,
                    in1=bc(bhh1n_s[:, :], [[1, 4], [0, NB]]), op=OP.add)
                nc.vector.tensor_mul(out=tn, in0=tn, in1=rz[:, 0:32])
                nc.vector.tensor_add(out=tn, in0=tn, in1=ig1_ps)
                nc.vector.tensor_add(out=tn, in0=tn, in1=gi1x_s[:, 64:96])
                n1 = sp.tile([128, 32], f32, tag="n1", name="n1")
                nc.scalar.activation(out=n1, in_=tn, func=AF.Tanh)
                dd = sp.tile([128, 32], f32, tag="dd", name="dd")
                nc.vector.tensor_sub(out=dd, in0=h1, in1=n1)
                nc.vector.tensor_mul(out=dd, in0=dd, in1=rz[:, 32:64])
                nc.vector.tensor_add(out=h1, in0=dd, in1=n1)
                nc.vector.tensor_copy(out=h1b, in_=h1)

                # proj + LN1
                mm_ws(proj_ps, proja_s, awe_s, 4, 4)
                x1 = sp.tile([128, 32], f32, tag="x1", name="x1")
                nc.vector.tensor_add(out=x1, in0=h1, in1=proj_ps)
                nc.vector.tensor_add(out=x1, in0=x1, in1=projx_s)

                h1r_bf = sp.tile([128, 32], bf16, tag="h1rbf", name="h1r_bf")
                h1r_f = sp.tile([128, 32], f32, tag="h1rf", name="h1r_f")
                layer_norm(x1, ln1g_s, ln1b_s, h1r_bf, h1r_f, "ln1", 0)

                # GRU2
                mm_gru(pB, wih2_s, h1r_bf, True)

                trz2 = sp.tile([128, 64], f32, tag="trz2", name="trz2")
                nc.vector.tensor_tensor(
                    out=trz2, in0=rz2_ps,
                    in1=bc(brz2_s[:, :], [[1, 8], [0, NB]]), op=OP.add)
                rz2 = sp.tile([128, 64], f32, tag="rz2", name="rz2")
                nc.scalar.activation(out=rz2, in_=trz2, func=AF.Sigmoid)
                tn2 = sp.tile([128, 32], f32, tag="tn2", name="tn2")
                nc.vector.tensor_tensor(
                    out=tn2, in0=hg2_ps,
                    in1=bc(bhh2n_s[:, :], [[1, 4], [0, NB]]), op=OP.add)
                nc.vector.tensor_mul(out=tn2, in0=tn2, in1=rz2[:, 0:32])
                nc.vector.tensor_add(out=tn2, in0=tn2, in1=ig2_ps)
                nc.vector.tensor_tensor(
                    out=tn2, in0=tn2,
                    in1=bc(bign2_s[:, :], [[1, 4], [0, NB]]), op=OP.add)
                n2 = sp.tile([128, 32], f32, tag="n2", name="n2")
                nc.scalar.activation(out=n2, in_=tn2, func=AF.Tanh)
                dd2 = sp.tile([128, 32], f32, tag="dd2", name="dd2")
                nc.vector.tensor_sub(out=dd2, in0=h2, in1=n2)
                nc.vector.tensor_mul(out=dd2, in0=dd2, in1=rz2[:, 32:64])
                nc.vector.tensor_add(out=h2, in0=dd2, in1=n2)
                nc.vector.tensor_copy(out=h2b, in_=h2)

                # LN2 -> h2res -> DRAM
                x2 = sp.tile([128, 32], f32, tag="x2", name="x2")
                nc.vector.tensor_add(out=x2, in0=h2, in1=h1r_f)
                h2r_bf = sp.tile([128, 32], bf16, tag="h2rbf", name="h2r_bf")
                layer_norm(x2, ln2g_s, ln2b_s, h2r_bf, None, "ln2", 128)
                nc.sync.dma_start(out=h2res_d[t, :, :], in_=h2r_bf)

            nc.sync.dma_start(out=sums[0:1, :], in_=sums_s)

        # --- FFN phase: rows r = t*8+b in blocks of 128
        n_blocks = T_PAD * NB // 128
        with tc.tile_pool(name="fp", bufs=3) as fp, \
             tc.tile_pool(name="fps", bufs=2, space="PSUM") as fps:
            for blk in range(n_blocks):
                t0 = blk * 16
                hin = fp.tile([128, 4 * 128], bf16, tag="hin", name="hin")
                for kt in range(4):
                    src = bass.AP(
                        tensor=h2res_d, offset=t0 * 128 * 32 + kt * 8,
                        ap=[[32, 128], [128 * 32, 16], [1, 8]])
                    nc.sync.dma_start(out=hin[:, kt * 128:(kt + 1) * 128],
                                      in_=src)
                relu1 = fp.tile([128, 8 * 128], bf16, tag="relu1", name="relu1")
                for mt in range(8):
                    f1_ps = fps.tile([128, 128], f32, tag="f_f1", name="f1_ps")
                    for kt in range(4):
                        nc.tensor.matmul(
                            f1_ps,
                            ffn1w_s[:, kt * FFN + mt * 128:
                                    kt * FFN + (mt + 1) * 128],
                            hin[:, kt * 128:(kt + 1) * 128],
                            start=(kt == 0), stop=(kt == 3))
                    nc.scalar.activation(out=relu1[:, mt * 128:(mt + 1) * 128],
                                         in_=f1_ps, func=AF.Relu,
                                         bias=ffn1b_s[:, mt:mt + 1])
                for vs in range(4):
                    f2_ps = fps.tile([128, 500], f32, tag="f_f2", name="f2_ps")
                    for kt in range(8):
                        nc.tensor.matmul(
                            f2_ps,
                            relu1[:, kt * 128:(kt + 1) * 128],
                            ffn2w_s[:, kt * V + vs * 500: kt * V + (vs + 1) * 500],
                            start=(kt == 0), stop=False)
                    nc.tensor.matmul(
                        f2_ps, ones_rb,
                        ffn2b_s[0:1, vs * 500:(vs + 1) * 500],
                        start=False, stop=True)
                    f2_sb = fp.tile([128, 500], f32, tag="f2sb", name="f2_sb")
                    nc.vector.tensor_copy(out=f2_sb, in_=f2_ps)
                    nc.sync.dma_start(
                        out=preds[blk * 128:(blk + 1) * 128,
                                  vs * 500:(vs + 1) * 500],
                        in_=f2_sb)

    _split_multi_waits(nc)
    return nc


# ---------------------------------------------------------------- host side
def _pack_lhsT(w, kts, m):
    """w [M, K] -> lhsT pack [128, kts*m] with [:, kt*m + j] = w[j, kt*128+p]."""
    K = kts * 128
    assert w.shape == (m, K)
    out = np.zeros((128, kts * m), np.float32)
    for kt in range(kts):
        out[:, kt * m:(kt + 1) * m] = w[:, kt * 128:(kt + 1) * 128].T
    return out


def _prep(inputs):
    c = {k: np.asarray(v) for k, v in inputs.items()}
    lens = c["caption_lengths"].astype(np.int64)
    order = np.argsort(-lens, kind="stable")
    enc = np.ascontiguousarray(c["encoder_out"][order]).astype(np.float32)
    caps = c["encoded_captions"][order]
    dec_lens = lens[order] - 1

    bf = np.dtype("bfloat16") if False else None
    import ml_dtypes
    tobf = lambda x: np.asarray(x, np.float32).astype(ml_dtypes.bfloat16)

    # x-side precompute (fp32 on host)
    xs = c["emb"].astype(np.float32)[caps[:, :T]]           # [B,T,EMB]
    wih = c["gru1_wih"].astype(np.float32)
    wih_x, wih_a = wih[:, :EMB], wih[:, EMB:]
    projw = c["proj_w"].astype(np.float32)
    proj_x, proj_a = projw[:, :EMB], projw[:, EMB:]
    bih1 = c["gru1_bih"].astype(np.float32)
    bhh1 = c["gru1_bhh"].astype(np.float32)
    bias_gi1 = bih1 + np.where(np.arange(3 * DEC) < 2 * DEC, bhh1, 0.0)
    gi1x = np.einsum("bte,de->btd", xs, wih_x, optimize=True) + bias_gi1
    projx = np.einsum("bte,de->btd", xs, proj_x, optimize=True) \
        + c["proj_b"].astype(np.float32)

    # weight packs (shared across cores)
    shared = {
        "attw": tobf(_pack_lhsT(c["dec_att_w"].astype(np.float32), 4, ATT)),
        "gatew1": tobf(_pack_lhsT(c["gate_w1"].astype(np.float32), 4, ATT)),
        "wiha": tobf(_pack_lhsT(wih_a, 4, 3 * DEC)),
        "whh1": tobf(_pack_lhsT(c["gru1_whh"].astype(np.float32), 4, 3 * DEC)),
        "wih2": tobf(_pack_lhsT(c["gru2_wih"].astype(np.float32), 4, 3 * DEC)),
        "whh2": tobf(_pack_lhsT(c["gru2_whh"].astype(np.float32), 4, 3 * DEC)),
        "proja": tobf(_pack_lhsT(proj_a, 4, DEC)),
        "encattw": tobf(_pack_lhsT(c["enc_att_w"].astype(np.float32), 4, ATT)),
        "ffn1w": tobf(_pack_lhsT(c["ffn_w1"].astype(np.float32), 4, FFN)),
        "wfull": tobf(c["full_att_w"].astype(np.float32).T),      # [128,1]
        "gatew2": tobf(c["gate_w2"].astype(np.float32).T),        # [128,1]
        "dec_att_b": c["dec_att_b"].astype(np.float32).reshape(128, 1),
        "enc_att_b": c["enc_att_b"].astype(np.float32).reshape(128, 1),
        "gate_b1": c["gate_b1"].astype(np.float32).reshape(128, 1),
        "gate_b2": c["gate_b2"].astype(np.float32).reshape(1, 1),
        "bhh1n": bhh1[2 * DEC:].reshape(4, 128).T.copy(),
        "ln1g": c["ln1_g"].astype(np.float32).reshape(4, 128).T.copy(),
        "ln1b": c["ln1_b"].astype(np.float32).reshape(4, 128).T.copy(),
        "ln2g": c["ln2_g"].astype(np.float32).reshape(4, 128).T.copy(),
        "ln2b": c["ln2_b"].astype(np.float32).reshape(4, 128).T.copy(),
    }
    bih2 = c["gru2_bih"].astype(np.float32)
    bhh2 = c["gru2_bhh"].astype(np.float32)
    brz2 = (bih2 + bhh2)[:2 * DEC]
    shared["brz2"] = brz2.reshape(8, 128).T.copy()
    shared["bign2"] = bih2[2 * DEC:].reshape(4, 128).T.copy()
    shared["bhh2n"] = bhh2[2 * DEC:].reshape(4, 128).T.copy()
    # ffn2w: [128, 8*V]: [p, kt*V+n] = ffn_w2[n, kt*128+p]
    f2 = c["ffn_w2"].astype(np.float32)  # [V, 1024]
    ffn2w = np.zeros((128, 8 * V), np.float32)
    for kt in range(8):
        ffn2w[:, kt * V:(kt + 1) * V] = f2[:, kt * 128:(kt + 1) * 128].T
    shared["ffn2w"] = tobf(ffn2w)
    shared["ffn1b"] = c["ffn_b1"].astype(np.float32).reshape(8, 128).T.copy()
    f2b = np.zeros((1, 2 * V), np.float32)
    f2b[0, :V] = c["ffn_b2"].astype(np.float32)
    shared["ffn2b"] = tobf(f2b)

    in_maps = []
    for core in range(NC_CORES):
        sl = slice(core * NB, (core + 1) * NB)
        e = enc[sl]                                   # [8,196,512]
        # encT [512, (b,196)]
        encT = np.ascontiguousarray(
            e.transpose(2, 0, 1).reshape(512, NB * P))
        # encP [98, (b, half, 512)]
        encP = np.ascontiguousarray(
            e.reshape(NB, 2, PH, ENC).transpose(2, 0, 1, 3).reshape(PH, NB * 2 * ENC))
        # gi1x [T_PAD, 128, 96]: [t, p, mt*8+b] = gi1x[b, t, mt*128+p]
        g = gi1x[sl]                                  # [8,T,1536]
        gcore = np.zeros((T_PAD, 128, 96), np.float32)
        gcore[:T] = g.reshape(NB, T, 12, 128).transpose(1, 3, 2, 0).reshape(T, 128, 96)
        p_ = projx[sl]
        pcore = np.zeros((T_PAD, 128, 32), np.float32)
        pcore[:T] = p_.reshape(NB, T, 4, 128).transpose(1, 3, 2, 0).reshape(T, 128, 32)
        m = dict(shared)
        m["encT"] = tobf(encT)
        m["encP"] = tobf(encP)
        m["gi1x"] = gcore
        m["projx"] = pcore
        in_maps.append(m)
    return in_maps, order, caps, dec_lens


def _assemble(results, order, caps, dec_lens, lens_dtype):
    preds_full = np.zeros((B, T, V), np.float32)
    alphas_full = np.zeros((B, T, P), np.float32)
    for core in range(NC_CORES):
        r = results[core]
        pr = r["preds"].reshape(T_PAD, NB, V)[:T]          # [T,8,V]
        ex = np.asarray(r["exout"], np.float32).reshape(T_PAD, PH, 2, NB)[:T]
        s = r["sums"].reshape(T_PAD * NB)[:T * NB].reshape(T, NB)
        al = ex.transpose(3, 0, 2, 1).reshape(NB, T, P)    # [b,t,(half,98)]
        al = al / s.T[:, :, None]
        for b_ in range(NB):
            gb = core * NB + b_
            L = int(dec_lens[gb])
            preds_full[gb, :L] = pr[:L, b_]
            alphas_full[gb, :L] = al[b_, :L]
    return preds_full, caps, dec_lens.astype(lens_dtype), alphas_full, \
        order.astype(lens_dtype)


def _patch_walrus():
    """birsim inside walrus dominates compile time for this 50k-instruction
    kernel; disable it (correctness is established against the reference)."""
    from concourse import bass_utils
    if getattr(bass_utils, "_nobirsim", False):
        return
    orig = bass_utils.bir_verify_and_optimise

    def patched(tmpdir, inp="bir.json", outp="file.neff", arch=None, *,
                dve_root=None):
        from pathlib import Path
        cmd = [
            bass_utils.get_walrus_driver(), "--pass",
            "birverifier,runtime_memory_reservation,lower_act,lower_dve,"
            "lower_ap_offset,codegen,neff_packager",
            "-i", inp, "--neff-output-filename", outp,
            "--enable-birsim=false", "--mem-mode=physical", "--policy=0",
            "--enable-ldw-opt=false", "--assign-static-dmas-to-sp=false",
            "--dram-page-size=256", "--enable-neff-debug-info=true",
            "--jobs", "8",
            *bass_utils.get_walrus_args(
                bass_utils.get_bir_arch(tmpdir, inp) if arch is None else arch,
                tmpdir, dve_root=dve_root),
        ]
        result = bass_utils.run_command(cmd, cwd=tmpdir)
        if result is not None:
            (Path(tmpdir) / "log.txt").write_text(result.stdout)
        return f"{tmpdir}/{outp}"

    bass_utils.bir_verify_and_optimise = patched
    import concourse.bass2jax  # ensure hook module sees patched fn via module attr
    bass_utils._nobirsim = True


def kernel(**inputs):
    import time, os
    _patch_walrus()
    from concourse.bass_utils import run_bass_kernel_spmd
    _t0 = time.time()
    if "nc" not in _CACHE:
        _CACHE["nc"] = _build(T)
    nc = _CACHE["nc"]
    _t1 = time.time()
    in_maps, order, caps, dec_lens = _prep(inputs)
    _t2 = time.time()
    res = run_bass_kernel_spmd(nc, in_maps, core_ids=list(range(NC_CORES)),
                               trace=bool(os.environ.get("KTRACE")))
    _t3 = time.time()
    _CACHE["exec_ns"] = res.exec_time_ns
    if os.environ.get("KVERBOSE"):
        print(f"[kernel] build {_t1-_t0:.1f}s prep {_t2-_t1:.1f}s "
              f"run {_t3-_t2:.1f}s exec_ns {res.exec_time_ns}", flush=True)
    lens_dtype = np.asarray(inputs["caption_lengths"]).dtype
    return _assemble(res.results, order, caps, dec_lens, lens_dtype)
